# revision 16
# baseline (speedup 1.0000x reference)
"""Trainium2 Bass kernel for the BDH fast-weight recurrent network.

Problem (see reference): for each batch element, a T=256-step recurrence with
  x_t   = L1norm(0.97*x_{t-1} + relu(v_t @ Dx^T))          (v_t = token_emb[idx_t])
  a*_t  = rho_{t-1} x_t ;  rho_t = 0.97*(rho_{t-1} + LN(v_t) x_t^T)
  y_t   = relu(LN(a*_t) @ Dy^T) * relu(x_t)
  out_t = LN(y_t @ E^T)

The kernel restructures this into feed-forward matmuls:
 - rho never materializes: a*_t = sum_{s<t} 0.97^{t-s} (x_s . x_t) LN(v_s)
   (decayed linear attention over the x sequence).
 - the x recurrence is linear given the per-step L1 scales S_t; since S_t ~ 100
   and eps=1e-6, S_t = sum(r_t) + 0.97 exactly in fp32, so X = G @ R with
   G[t,s] = 0.97^{t-s} / prod_{j=s..t} S_j.  G factors as
   gexp[s,t] * P_{s-1} / P_t with P_t = prod_{j<=t} (S_j/100) (range ~1, fp32
   safe) and gexp = host-precomputed exp part.  P is a prefix product done
   with a DVE scan -- no Ln/Exp activations anywhere in the kernel.
 - X carries a constant 2^8 factor (from the fp16-range shift in gexp).
 - the a* layernorm's 1/(std+eps) factor is positively homogeneous through
   relu(LN(a*) @ Dy^T), so a* is only mean-centered; the missing per-row
   factor is folded into a per-row eps of the output layernorm.

All scalar-engine activations are Relu/Sqrt/Copy (one table set, no reloads).
Emission is software-pipelined: batch b+1's front half (gather/U/R/G) is
emitted between batch b's XT and AT stages so the scheduler can fill the
tensor engine during b's serial LN/scan chains.

Sharding: data-parallel over batch, 4 sequences per NeuronCore x 8 cores,
no cross-core communication.
"""

import sys

if "/opt/trn_rl_repo" not in sys.path:
    sys.path.insert(0, "/opt/trn_rl_repo")

import numpy as np

import concourse.bass as bass
import concourse.bacc as bacc
import concourse.tile as tile
from concourse import mybir
from concourse.bass_utils import run_bass_kernel_spmd

AF = mybir.ActivationFunctionType
OP = mybir.AluOpType

N, D, V = 4096, 256, 32000
B, T = 32, 256
BL = 4              # batch per core
NCORES = 8
XD = 0.97           # x decay
UD = 0.97           # rho decay
EPS = 1e-6
MU = float(np.log(100.0))
LNXD = float(np.log(XD))

F32 = mybir.dt.float32
F16 = mybir.dt.float16
MODE = "f16"                 # "f32" | "f32r" | "f16" for the large matmuls
MODE_DT = {"f32": mybir.dt.float32, "f32r": mybir.dt.float32r,
           "f16": mybir.dt.float16}
MM_DT = MODE_DT[MODE]
GT_LOG_SCALE = 8.0 * float(np.log(2.0))   # store GT * 2^8 (fp16 underflow guard)
# out-LN eps: eps * c_t with c_t = 2^24*(s_a + 1e-6), s_a = std(a*) = 2^-16*s~_a
EPSV_COEF = 256.0 * EPS      # * s~_a (the 2^16-scaled a* std we measure)
EPSV_CONST = 2.0 ** 24 * EPS * EPS

NT = N // 128       # 32 n tiles
TT = T // 128       # 2 t tiles
DT = D // 128       # 2 d tiles
DDOF = float(D) / (D - 1)


def _host_consts():
    """Constant tensors shipped to every core (computed in float64, cast f32)."""
    si = np.arange(T, dtype=np.float64)[:, None]
    ti = np.arange(T, dtype=np.float64)[None, :]
    k = ti - si
    kconst = np.where(k >= 0, k * LNXD - (k + 1) * MU + GT_LOG_SCALE, -np.inf)
    gexp = np.exp(kconst).astype(np.float32)          # banded: underflow -> 0
    gexp = gexp.reshape(TT, 128, T).transpose(1, 0, 2)
    decayT = np.where(k > 0, UD ** np.maximum(k, 0.0), 0.0)
    decayT = decayT.astype(np.float32).reshape(TT, 128, T).transpose(1, 0, 2)
    svb = np.full((T,), XD, np.float32)
    svb[0] = 0.0
    svb = svb.reshape(TT, 128).T.copy()
    return {
        "gexp": np.ascontiguousarray(gexp),       # (128, TT, T)
        "decayT": np.ascontiguousarray(decayT),   # (128, TT, T)
        "svb": np.ascontiguousarray(svb),         # (128, TT)
    }


def _ln_stats(nc, tiny, z_in):
    """bn_stats/bn_aggr: returns mv tile [128, 2] = (mean, var_pop)."""
    st6 = tiny.tile([128, 6], F32, tag="ln_st6")
    mv = tiny.tile([128, 2], F32, tag="ln_mv")
    nc.vector.bn_stats(out=st6[:], in_=z_in)
    nc.vector.bn_aggr(out=mv[:], in_=st6[:])
    return mv


def _ln_row(nc, tiny, z_in, out_ap, eps, eps_ap=None):
    """LayerNorm over the free dim (size D): out = (z - m)/(std_ddof1 + eps).

    z_in may be a PSUM or SBUF AP of shape (128, D).  Sqrt-based (same scalar
    table set as Relu).  eps_ap optionally adds a per-row eps term.
    """
    mv = _ln_stats(nc, tiny, z_in)
    s = tiny.tile([128, 1], F32, tag="ln_s")
    nc.scalar.activation(out=s[:], in_=mv[:, 1:2], func=AF.Sqrt, scale=DDOF)
    if eps_ap is not None:
        nc.vector.tensor_tensor(out=s[:], in0=s[:], in1=eps_ap, op=OP.add)
    if eps:
        nc.vector.tensor_scalar(out=s[:], in0=s[:], scalar1=eps, scalar2=None,
                                op0=OP.add)
    recip = tiny.tile([128, 1], F32, tag="ln_recip")
    nc.vector.reciprocal(out=recip[:], in_=s[:])
    nc.vector.tensor_scalar(
        out=out_ap, in0=z_in, scalar1=mv[:, 0:1], scalar2=recip[:],
        op0=OP.subtract, op1=OP.mult,
    )


def build_nc(mm_dt=MM_DT, dbg=False, dbg_keys=None):
    nc = bacc.Bacc("TRN2", target_bir_lowering=False, debug=False)

    idx_d = nc.dram_tensor("idx", [BL * T], mybir.dt.int32, kind="ExternalInput").ap()
    temb_d = nc.dram_tensor("temb", [V, D], F16, kind="ExternalInput").ap()
    dxt_d = nc.dram_tensor("dxt", [D, N], mm_dt, kind="ExternalInput").ap()
    dyt_d = nc.dram_tensor("dyt", [D, N], mm_dt, kind="ExternalInput").ap()
    et_d = nc.dram_tensor("et", [N, D], mm_dt, kind="ExternalInput").ap()
    gexp_d = nc.dram_tensor("gexp", [128, TT, T], F32, kind="ExternalInput").ap()
    decayT_d = nc.dram_tensor("decayT", [128, TT, T], F32, kind="ExternalInput").ap()
    svb_d = nc.dram_tensor("svb", [128, TT], F32, kind="ExternalInput").ap()
    identh_d = nc.dram_tensor("identh", [128, 128], F16, kind="ExternalInput").ap()
    identf_d = nc.dram_tensor("identf", [128, 128], F32, kind="ExternalInput").ap()
    esel_d = nc.dram_tensor("esel", [2, TT, 128], F32, kind="ExternalInput").ap()
    out_d = nc.dram_tensor("out", [BL, T, D], F32, kind="ExternalOutput").ap()

    with tile.TileContext(nc) as tc:
        with (
            tc.tile_pool(name="consts", bufs=1) as consts,
            tc.tile_pool(name="big", bufs=2) as big,
            tc.tile_pool(name="mid", bufs=2) as mid,
            tc.tile_pool(name="tiny", bufs=6) as tiny,
            tc.tile_pool(name="scratch", bufs=4) as scratch,
            tc.tile_pool(name="vpool", bufs=2) as vpool,
            tc.tile_pool(name="psA", bufs=6, space="PSUM") as psA,
            tc.tile_pool(name="psS", bufs=2, space="PSUM") as psS,
        ):
            # ---- idx + embedding gathers first: tiny, and they gate the
            # first transposes/U/R -- don't starve them behind the 6.8MB
            # weight stream ----
            idx_t = consts.tile([128, 2 * BL], mybir.dt.int32)
            nc.sync.dma_start(out=idx_t[:], in_=idx_d.rearrange("(j p) -> p j", p=128))
            vprevs = {}

            def stage_gather(b):
                vprev = vpool.tile([128, TT, D], F16, tag="vprev")
                vprevs[b] = vprev
                for m in range(TT):
                    nc.gpsimd.indirect_dma_start(
                        out=vprev[:, m, :],
                        out_offset=None,
                        in_=temb_d[:],
                        in_offset=bass.IndirectOffsetOnAxis(
                            ap=idx_t[:, TT * b + m : TT * b + m + 1], axis=0
                        ),
                    )

            stage_gather(0)
            stage_gather(1)
            identh = consts.tile([128, 128], F16)
            nc.sync.dma_start(out=identh[:], in_=identh_d[:])
            identf = consts.tile([128, 128], F32)
            nc.sync.dma_start(out=identf[:], in_=identf_d[:])
            dxt = consts.tile([128, DT, N], mm_dt)
            dxt_src = dxt_d.rearrange("(k p) n -> p k n", p=128)
            for kd in range(DT):
                nc.sync.dma_start(out=dxt[:, kd, :], in_=dxt_src[:, kd, :])
            svb = consts.tile([128, TT], F32)
            nc.sync.dma_start(out=svb[:], in_=svb_d[:])
            gexp = consts.tile([128, TT, T], F32)
            nc.sync.dma_start(out=gexp[:], in_=gexp_d[:])
            # esel[p, j, :] = 1.0 if p == j else 0.0  (row selectors for PE
            # partition-broadcast: out = esel[:,j,:].T @ rows picks row j)
            esel = consts.tile([2, TT, 128], F32)
            nc.sync.dma_start(out=esel[:], in_=esel_d[:])

            # late constants, emitted after batch 0's front half below
            decayT = consts.tile([128, TT, T], F32)
            dyt = consts.tile([128, DT, N], mm_dt)
            et = consts.tile([128, NT, D], mm_dt)

            def emit_late_consts():
                nc.sync.dma_start(out=decayT[:], in_=decayT_d[:])
                dyt_src = dyt_d.rearrange("(k p) n -> p k n", p=128)
                for kd in range(DT):
                    nc.sync.dma_start(out=dyt[:, kd, :], in_=dyt_src[:, kd, :])
                et_src = et_d.rearrange("(k p) d -> p k d", p=128)
                for kq in range(4):
                    nc.sync.dma_start(out=et[:, kq * 8 : (kq + 1) * 8, :],
                                      in_=et_src[:, kq * 8 : (kq + 1) * 8, :])

            tiles = {}

            def stage_front(b):
                """U = LN(v), R = relu(v@DxT), G factors -> GT."""
                if b not in vprevs:
                    stage_gather(b)
                vprev = vprevs.pop(b)
                vprevT = mid.tile([128, DT, T], mm_dt, tag="vprevT")
                U = mid.tile([128, TT, D], mm_dt, tag="U")
                for m in range(TT):
                    for kd in range(DT):
                        pt = psS.tile([128, 128], F16, tag="pss")
                        nc.tensor.transpose(
                            out=pt[:], in_=vprev[:, m, kd * 128 : (kd + 1) * 128],
                            identity=identh[:],
                        )
                        nc.vector.tensor_copy(
                            out=vprevT[:, kd, m * 128 : (m + 1) * 128], in_=pt[:]
                        )
                    _ln_row(nc, tiny, vprev[:, m, :], U[:, m, :], EPS)

                R = big.tile([128, TT, N], mm_dt, tag="R")
                rs = tiny.tile([128, TT, 8], F32, tag="rs")
                for m in range(TT):
                    for nq in range(8):
                        pr = psA.tile([128, 512], F32, tag="psa")
                        for kd in range(DT):
                            nc.tensor.matmul(
                                pr[:],
                                vprevT[:, kd, m * 128 : (m + 1) * 128],
                                dxt[:, kd, nq * 512 : (nq + 1) * 512],
                                start=(kd == 0),
                                stop=(kd == DT - 1),
                            )
                        nc.scalar.activation(
                            out=R[:, m, nq * 512 : (nq + 1) * 512], in_=pr[:],
                            func=AF.Relu, accum_out=rs[:, m, nq : nq + 1],
                        )

                # G factors: q = (S/100), prefix products P via DVE scan
                q = tiny.tile([128, TT], F32, tag="q")
                for m in range(TT):
                    rsum = tiny.tile([128, 1], F32, tag="rsum")
                    nc.vector.tensor_reduce(
                        out=rsum[:], in_=rs[:, m, :], axis=mybir.AxisListType.X, op=OP.add
                    )
                    nc.vector.tensor_scalar(
                        out=q[:, m : m + 1], in0=rsum[:], scalar1=svb[:, m : m + 1],
                        scalar2=0.01, op0=OP.add, op1=OP.mult,
                    )
                # q is on t-partitions; move to a row, scan for prefix
                # products P, and bring P and 1/q back to partitions with a
                # single full-width transpose (rows 0:2 = P, rows 2:4 = 1/q;
                # u = P_{s-1} = P_s * (1/q_s) applied as two scalar factors)
                pq = psS.tile([TT, 128], F32, tag="pss")
                nc.tensor.transpose(out=pq[:], in_=q[:], identity=identf[:])
                qrow = tiny.tile([TT, 128], F32, tag="qrow")
                nc.vector.tensor_copy(out=qrow[:], in_=pq[:])
                pad = scratch.tile([128, 128], F32, tag="pm1pad")
                nc.vector.tensor_tensor_scan(
                    out=pad[0:2, :], data0=qrow[:], data1=qrow[:], initial=1.0,
                    op0=OP.mult, op1=OP.bypass,
                )
                nc.vector.reciprocal(out=pad[32:34, :], in_=qrow[:])
                rP = tiny.tile([TT, 128], F32, tag="rP")
                nc.vector.reciprocal(out=rP[:], in_=pad[0:2, :])
                pb = psS.tile([128, 128], F32, tag="pss")
                nc.tensor.transpose(out=pb[:], in_=pad[:], identity=identf[:])
                Pq4 = tiny.tile([128, 4], F32, tag="Pq4")
                nc.vector.tensor_copy(out=Pq4[:, 0:2], in_=pb[:, 0:2])
                nc.vector.tensor_copy(out=Pq4[:, 2:4], in_=pb[:, 32:34])
                ptb = psS.tile([128, 1], F32, tag="pss")
                nc.tensor.matmul(ptb[:], esel[:, 0, :], rP[:, 127:128],
                                 start=True, stop=True)
                # ucross = P_{s-1}(tile0) / Ptot0 = ptb * P_s * (1/q_s)
                ucross = tiny.tile([128, 1], F32, tag="ucross")
                nc.vector.tensor_scalar(
                    out=ucross[:], in0=ptb[:], scalar1=Pq4[:, 0:1],
                    scalar2=Pq4[:, 2:3], op0=OP.mult, op1=OP.mult,
                )
                GT = mid.tile([128, TT, T], mm_dt, tag="GT")
                for tau in range(TT):
                    pw = psS.tile([128, 128], F32, tag="pss")
                    nc.tensor.matmul(pw[:], esel[:, tau, :], rP[:, :],
                                     start=True, stop=True)
                    for m in range(TT):
                        if tau == 0 and m == 1:
                            continue
                        tmp = scratch.tile([128, 128], F32, tag="gt_tmp")
                        if tau == 1 and m == 0:
                            nc.vector.tensor_scalar(
                                out=tmp[:], in0=gexp[:, m, 128:256],
                                scalar1=ucross[:], scalar2=None, op0=OP.mult,
                            )
                        else:
                            nc.vector.tensor_scalar(
                                out=tmp[:], in0=gexp[:, m, tau * 128 : (tau + 1) * 128],
                                scalar1=Pq4[:, m : m + 1],
                                scalar2=Pq4[:, 2 + m : 3 + m],
                                op0=OP.mult, op1=OP.mult,
                            )
                        nc.vector.tensor_tensor(
                            out=GT[:, m, tau * 128 : (tau + 1) * 128],
                            in0=tmp[:], in1=pw[:], op=OP.mult,
                        )
                tiles[b] = {"U": U, "R": R, "GT": GT}

            def stage_xt(b):
                """X^T = R^T @ G^T (2^8 scale); s-tile 1 only feeds t >= 128."""
                t = tiles[b]
                XT = big.tile([128, NT, T], mm_dt, tag="XT")
                t["XT"] = XT
                R, GT = t["R"], t["GT"]
                for ni in range(NT // 2):
                    px = psA.tile([128, 2, 256], F32, tag="psa")
                    for h in range(2):
                        nt = 2 * ni + h
                        nc.tensor.matmul(
                            px[:, h, 0:144],
                            R[:, 0, nt * 128 : (nt + 1) * 128], GT[:, 0, 0:144],
                            start=True, stop=False,
                        )
                        nc.tensor.matmul(
                            px[:, h, 128:256],
                            R[:, 1, nt * 128 : (nt + 1) * 128], GT[:, 1, 128:],
                            start=False, stop=True, skip_group_check=True,
                        )
                    nc.vector.tensor_copy(
                        out=XT[:, 2 * ni : 2 * ni + 2, :], in_=px[:]
                    )

            def stage_back(b):
                """Scores/AT, centered a*, y, v* = LN(y@E^T) -> out."""
                t = tiles.pop(b)
                XT, U = t["XT"], t["U"]

                AT = mid.tile([128, TT, T], mm_dt, tag="AT")
                for st in range(TT):
                    lo = st * 128
                    psc = psA.tile([128, T - lo], F32, tag="psa")
                    for k in range(NT):
                        nc.tensor.matmul(
                            psc[:], XT[:, k, lo : lo + 128], XT[:, k, lo:],
                            start=(k == 0), stop=(k == NT - 1),
                        )
                    nc.vector.tensor_tensor(
                        out=AT[:, st, lo:], in0=psc[:], in1=decayT[:, st, lo:], op=OP.mult
                    )

                ynorm = mid.tile([128, TT, D], F16, tag="ynorm")
                ynormT = mid.tile([128, DT, T], mm_dt, tag="ynormT")
                epsv = tiny.tile([128, TT], F32, tag="epsv")
                for tt in range(TT):
                    pa = psA.tile([128, D], F32, tag="psa")
                    for k in range(tt + 1):
                        nc.tensor.matmul(
                            pa[:], AT[:, k, tt * 128 : (tt + 1) * 128], U[:, k, :],
                            start=(k == 0), stop=(k == tt),
                        )
                    mv = _ln_stats(nc, tiny, pa[:])
                    sa = tiny.tile([128, 1], F32, tag="sa")
                    nc.scalar.activation(out=sa[:], in_=mv[:, 1:2], func=AF.Sqrt,
                                         scale=DDOF)
                    nc.vector.tensor_scalar(
                        out=epsv[:, tt : tt + 1], in0=sa[:], scalar1=EPSV_COEF,
                        scalar2=EPSV_CONST, op0=OP.mult, op1=OP.add,
                    )
                    nc.vector.tensor_scalar(
                        out=ynorm[:, tt, :], in0=pa[:], scalar1=mv[:, 0:1],
                        scalar2=None, op0=OP.subtract,
                    )
                    for kd in range(DT):
                        pt = psS.tile([128, 128], F16, tag="pss")
                        nc.tensor.transpose(
                            out=pt[:], in_=ynorm[:, tt, kd * 128 : (kd + 1) * 128],
                            identity=identh[:],
                        )
                        nc.vector.tensor_copy(
                            out=ynormT[:, kd, tt * 128 : (tt + 1) * 128], in_=pt[:]
                        )

                for ni in range(NT // 2):
                    py = psA.tile([128, 2, 256], F32, tag="psa")
                    for h in range(2):
                        nt = 2 * ni + h
                        for kd in range(DT):
                            nc.tensor.matmul(
                                py[:, h, :],
                                dyt[:, kd, nt * 128 : (nt + 1) * 128],
                                ynormT[:, kd, :],
                                start=(kd == 0), stop=(kd == DT - 1),
                            )
                    yr = scratch.tile([128, 2, 256], mm_dt, tag="yrelu")
                    nc.scalar.activation(out=yr[:], in_=py[:], func=AF.Relu)
                    xv = XT[:, 2 * ni : 2 * ni + 2, :]
                    nc.vector.tensor_tensor(out=xv, in0=yr[:], in1=xv, op=OP.mult)

                for tt in range(TT):
                    pv = psA.tile([128, D], F32, tag="psa")
                    for k in range(NT):
                        nc.tensor.matmul(
                            pv[:], XT[:, k, tt * 128 : (tt + 1) * 128], et[:, k, :],
                            start=(k == 0), stop=(k == NT - 1),
                        )
                    vstar = scratch.tile([128, D], F32, tag="vstar")
                    _ln_row(nc, tiny, pv[:], vstar[:], 0.0,
                            eps_ap=epsv[:, tt : tt + 1])
                    nc.sync.dma_start(
                        out=out_d[b, tt * 128 : (tt + 1) * 128, :], in_=vstar[:]
                    )

            # ---- software-pipelined emission ----
            stage_front(0)
            emit_late_consts()
            stage_front(1)
            for b in range(BL):
                stage_xt(b)
                stage_back(b)
                if b + 2 < BL:
                    stage_front(b + 2)

    nc.compile()
    return nc


_NC_CACHE = {}


def _get_nc(mm_dt=MM_DT):
    key = str(mm_dt)
    if key not in _NC_CACHE:
        _NC_CACHE[key] = build_nc(mm_dt)
    return _NC_CACHE[key]


def make_in_maps(idx, token_emb, E, Dx, Dy, mm_dt=MM_DT):
    wdt = mybir.dt.np(mm_dt)
    idx = np.ascontiguousarray(np.asarray(idx).astype(np.int32))
    temb = np.ascontiguousarray(np.asarray(token_emb, np.float32).astype(np.float16))
    dxt = np.ascontiguousarray(np.asarray(Dx, np.float32).T.astype(wdt))
    dyt = np.ascontiguousarray(np.asarray(Dy, np.float32).T.astype(wdt))
    et = np.ascontiguousarray(np.asarray(E, np.float32).T.astype(wdt))
    consts = _host_consts()
    identh = np.ascontiguousarray(np.eye(128, dtype=np.float16))
    identf = np.ascontiguousarray(np.eye(128, dtype=np.float32))
    esel = np.zeros((2, TT, 128), np.float32)
    for j in range(TT):
        esel[j, j, :] = 1.0
    shared = {"temb": temb, "dxt": dxt, "dyt": dyt, "et": et,
              "identh": identh, "identf": identf,
              "esel": np.ascontiguousarray(esel), **consts}
    in_maps = []
    for c in range(NCORES):
        m = dict(shared)
        m["idx"] = np.ascontiguousarray(idx[c * BL : (c + 1) * BL].reshape(-1))
        in_maps.append(m)
    return in_maps


def kernel(idx, token_emb, E, Dx, Dy):
    nc = _get_nc()
    in_maps = make_in_maps(idx, token_emb, E, Dx, Dy)
    res = run_bass_kernel_spmd(nc, in_maps, core_ids=list(range(NCORES)))
    out = np.concatenate([r["out"] for r in res.results], axis=0)
    return out


# revision 17
# speedup vs baseline: 1.2007x; 1.2007x over previous
"""Trainium2 Bass kernel for the BDH fast-weight recurrent network.

Problem (see reference): for each batch element, a T=256-step recurrence with
  x_t   = L1norm(0.97*x_{t-1} + relu(v_t @ Dx^T))          (v_t = token_emb[idx_t])
  a*_t  = rho_{t-1} x_t ;  rho_t = 0.97*(rho_{t-1} + LN(v_t) x_t^T)
  y_t   = relu(LN(a*_t) @ Dy^T) * relu(x_t)
  out_t = LN(y_t @ E^T)

The kernel restructures this into feed-forward matmuls:
 - rho never materializes: a*_t = sum_{s<t} 0.97^{t-s} (x_s . x_t) LN(v_s)
   (decayed linear attention over the x sequence).
 - the x recurrence is linear given the per-step L1 scales S_t; since S_t ~ 100
   and eps=1e-6, S_t = sum(r_t) + 0.97 exactly in fp32, so X = G @ R with
   G[t,s] = 0.97^{t-s} / prod_{j=s..t} S_j.  G factors as
   gexp[s,t] * P_{s-1} / P_t with P_t = prod_{j<=t} (S_j/100) (range ~1, fp32
   safe) and gexp = host-precomputed exp part.  P is a prefix product done
   with a DVE scan -- no Ln/Exp activations anywhere in the kernel.
 - X carries a constant 2^8 factor (from the fp16-range shift in gexp).
 - the a* layernorm's 1/(std+eps) factor is positively homogeneous through
   relu(LN(a*) @ Dy^T), so a* is only mean-centered; the missing per-row
   factor is folded into a per-row eps of the output layernorm.

All scalar-engine activations are Relu/Sqrt/Copy (one table set, no reloads).
Emission is software-pipelined: batch b+1's front half (gather/U/R/G) is
emitted between batch b's XT and AT stages so the scheduler can fill the
tensor engine during b's serial LN/scan chains.

Sharding: data-parallel over batch, 4 sequences per NeuronCore x 8 cores,
no cross-core communication.
"""

import sys

if "/opt/trn_rl_repo" not in sys.path:
    sys.path.insert(0, "/opt/trn_rl_repo")

import numpy as np

import concourse.bass as bass
import concourse.bacc as bacc
import concourse.tile as tile
from concourse import mybir
from concourse.bass_utils import run_bass_kernel_spmd

AF = mybir.ActivationFunctionType
OP = mybir.AluOpType

N, D, V = 4096, 256, 32000
B, T = 32, 256
BL = 4              # batch per core
NCORES = 8
XD = 0.97           # x decay
UD = 0.97           # rho decay
EPS = 1e-6
MU = float(np.log(100.0))
LNXD = float(np.log(XD))

F32 = mybir.dt.float32
F16 = mybir.dt.float16
MODE = "f16"                 # "f32" | "f32r" | "f16" for the large matmuls
MODE_DT = {"f32": mybir.dt.float32, "f32r": mybir.dt.float32r,
           "f16": mybir.dt.float16}
MM_DT = MODE_DT[MODE]
GT_LOG_SCALE = 8.0 * float(np.log(2.0))   # store GT * 2^8 (fp16 underflow guard)
# out-LN eps: eps * c_t with c_t = 2^24*(s_a + 1e-6), s_a = std(a*) = 2^-16*s~_a
EPSV_COEF = 256.0 * EPS      # * s~_a (the 2^16-scaled a* std we measure)
EPSV_CONST = 2.0 ** 24 * EPS * EPS

NT = N // 128       # 32 n tiles
TT = T // 128       # 2 t tiles
DT = D // 128       # 2 d tiles
DDOF = float(D) / (D - 1)


def _host_consts():
    """Constant tensors shipped to every core (computed in float64, cast f32)."""
    si = np.arange(T, dtype=np.float64)[:, None]
    ti = np.arange(T, dtype=np.float64)[None, :]
    k = ti - si
    kconst = np.where(k >= 0, k * LNXD - (k + 1) * MU + GT_LOG_SCALE, -np.inf)
    gexp = np.exp(kconst).astype(np.float32)          # banded: underflow -> 0
    gexp = gexp.reshape(TT, 128, T).transpose(1, 0, 2)
    decayT = np.where(k > 0, UD ** np.maximum(k, 0.0), 0.0)
    decayT = decayT.astype(np.float32).reshape(TT, 128, T).transpose(1, 0, 2)
    svb = np.full((T,), XD, np.float32)
    svb[0] = 0.0
    svb = svb.reshape(TT, 128).T.copy()
    return {
        "gexp": np.ascontiguousarray(gexp),       # (128, TT, T)
        "decayT": np.ascontiguousarray(decayT),   # (128, TT, T)
        "svb": np.ascontiguousarray(svb),         # (128, TT)
    }


def _ln_stats(nc, tiny, z_in):
    """bn_stats/bn_aggr: returns mv tile [128, 2] = (mean, var_pop)."""
    st6 = tiny.tile([128, 6], F32, tag="ln_st6")
    mv = tiny.tile([128, 2], F32, tag="ln_mv")
    nc.vector.bn_stats(out=st6[:], in_=z_in)
    nc.vector.bn_aggr(out=mv[:], in_=st6[:])
    return mv


def _ln_row(nc, tiny, z_in, out_ap, eps, eps_ap=None):
    """LayerNorm over the free dim (size D): out = (z - m)/(std_ddof1 + eps).

    z_in may be a PSUM or SBUF AP of shape (128, D).  Sqrt-based (same scalar
    table set as Relu).  eps_ap optionally adds a per-row eps term.
    """
    mv = _ln_stats(nc, tiny, z_in)
    s = tiny.tile([128, 1], F32, tag="ln_s")
    nc.scalar.activation(out=s[:], in_=mv[:, 1:2], func=AF.Sqrt, scale=DDOF)
    if eps_ap is not None:
        nc.vector.tensor_tensor(out=s[:], in0=s[:], in1=eps_ap, op=OP.add)
    if eps:
        nc.vector.tensor_scalar(out=s[:], in0=s[:], scalar1=eps, scalar2=None,
                                op0=OP.add)
    recip = tiny.tile([128, 1], F32, tag="ln_recip")
    nc.vector.reciprocal(out=recip[:], in_=s[:])
    nc.vector.tensor_scalar(
        out=out_ap, in0=z_in, scalar1=mv[:, 0:1], scalar2=recip[:],
        op0=OP.subtract, op1=OP.mult,
    )


def build_nc(mm_dt=MM_DT, dbg=False, dbg_keys=None):
    nc = bacc.Bacc("TRN2", target_bir_lowering=False, debug=False)

    idx_d = nc.dram_tensor("idx", [BL * T], mybir.dt.int32, kind="ExternalInput").ap()
    temb_d = nc.dram_tensor("temb", [V, D], F16, kind="ExternalInput").ap()
    dxt_d = nc.dram_tensor("dxt", [D, N], mm_dt, kind="ExternalInput").ap()
    dyt_d = nc.dram_tensor("dyt", [D, N], mm_dt, kind="ExternalInput").ap()
    et_d = nc.dram_tensor("et", [N, D], mm_dt, kind="ExternalInput").ap()
    gexp_d = nc.dram_tensor("gexp", [128, TT, T], F32, kind="ExternalInput").ap()
    decayT_d = nc.dram_tensor("decayT", [128, TT, T], F32, kind="ExternalInput").ap()
    svb_d = nc.dram_tensor("svb", [128, TT], F32, kind="ExternalInput").ap()
    identh_d = nc.dram_tensor("identh", [128, 128], F16, kind="ExternalInput").ap()
    identf_d = nc.dram_tensor("identf", [128, 128], F32, kind="ExternalInput").ap()
    esel_d = nc.dram_tensor("esel", [2, TT, 128], F32, kind="ExternalInput").ap()
    out_d = nc.dram_tensor("out", [BL, T, D], F32, kind="ExternalOutput").ap()

    with tile.TileContext(nc) as tc:
        with (
            tc.tile_pool(name="consts", bufs=1) as consts,
            tc.tile_pool(name="big", bufs=2) as big,
            tc.tile_pool(name="mid", bufs=2) as mid,
            tc.tile_pool(name="tiny", bufs=6) as tiny,
            tc.tile_pool(name="scratch", bufs=4) as scratch,
            tc.tile_pool(name="vpool", bufs=2) as vpool,
            tc.tile_pool(name="psA", bufs=6, space="PSUM") as psA,
            tc.tile_pool(name="psS", bufs=2, space="PSUM") as psS,
        ):
            # ---- idx + embedding gathers first: tiny, and they gate the
            # first transposes/U/R -- don't starve them behind the 6.8MB
            # weight stream ----
            idx_t = consts.tile([128, 2 * BL], mybir.dt.int32)
            nc.sync.dma_start(out=idx_t[:], in_=idx_d.rearrange("(j p) -> p j", p=128))
            vprevs = {}

            def stage_gather(b):
                vprev = vpool.tile([128, TT, D], F16, tag="vprev")
                vprevs[b] = vprev
                for m in range(TT):
                    nc.gpsimd.indirect_dma_start(
                        out=vprev[:, m, :],
                        out_offset=None,
                        in_=temb_d[:],
                        in_offset=bass.IndirectOffsetOnAxis(
                            ap=idx_t[:, TT * b + m : TT * b + m + 1], axis=0
                        ),
                    )

            stage_gather(0)
            stage_gather(1)
            identh = consts.tile([128, 128], F16)
            nc.sync.dma_start(out=identh[:], in_=identh_d[:])
            identf = consts.tile([128, 128], F32)
            nc.sync.dma_start(out=identf[:], in_=identf_d[:])
            dxt = consts.tile([128, DT, N], mm_dt)
            dxt_src = dxt_d.rearrange("(k p) n -> p k n", p=128)
            for kd in range(DT):
                nc.sync.dma_start(out=dxt[:, kd, :], in_=dxt_src[:, kd, :])
            svb = consts.tile([128, TT], F32)
            nc.sync.dma_start(out=svb[:], in_=svb_d[:])
            gexp = consts.tile([128, TT, T], F32)
            nc.sync.dma_start(out=gexp[:], in_=gexp_d[:])
            # esel[p, j, :] = 1.0 if p == j else 0.0  (row selectors for PE
            # partition-broadcast: out = esel[:,j,:].T @ rows picks row j)
            esel = consts.tile([2, TT, 128], F32)
            nc.sync.dma_start(out=esel[:], in_=esel_d[:])

            # late constants, emitted after batch 0's front half below
            decayT = consts.tile([128, TT, T], F32)
            dyt = consts.tile([128, DT, N], mm_dt)
            et = consts.tile([128, NT, D], mm_dt)

            def emit_late_consts():
                nc.sync.dma_start(out=decayT[:], in_=decayT_d[:])
                dyt_src = dyt_d.rearrange("(k p) n -> p k n", p=128)
                for kd in range(DT):
                    nc.sync.dma_start(out=dyt[:, kd, :], in_=dyt_src[:, kd, :])
                et_src = et_d.rearrange("(k p) d -> p k d", p=128)
                for kq in range(4):
                    nc.sync.dma_start(out=et[:, kq * 8 : (kq + 1) * 8, :],
                                      in_=et_src[:, kq * 8 : (kq + 1) * 8, :])

            tiles = {}

            def stage_front(b):
                """U = LN(v), R = relu(v@DxT), G factors -> GT."""
                if b not in vprevs:
                    stage_gather(b)
                vprev = vprevs.pop(b)
                vprevT = mid.tile([128, DT, T], mm_dt, tag="vprevT")
                U = mid.tile([128, TT, D], mm_dt, tag="U")
                for m in range(TT):
                    for kd in range(DT):
                        pt = psS.tile([128, 128], F16, tag="pss")
                        nc.tensor.transpose(
                            out=pt[:], in_=vprev[:, m, kd * 128 : (kd + 1) * 128],
                            identity=identh[:],
                        )
                        nc.vector.tensor_copy(
                            out=vprevT[:, kd, m * 128 : (m + 1) * 128], in_=pt[:]
                        )
                    _ln_row(nc, tiny, vprev[:, m, :], U[:, m, :], EPS)

                R = big.tile([128, TT, N], mm_dt, tag="R")
                rs = tiny.tile([128, TT, 8], F32, tag="rs")
                for m in range(TT):
                    for nq in range(8):
                        pr = psA.tile([128, 512], F32, tag="psa")
                        for kd in range(DT):
                            nc.tensor.matmul(
                                pr[:],
                                vprevT[:, kd, m * 128 : (m + 1) * 128],
                                dxt[:, kd, nq * 512 : (nq + 1) * 512],
                                start=(kd == 0),
                                stop=(kd == DT - 1),
                            )
                        nc.scalar.activation(
                            out=R[:, m, nq * 512 : (nq + 1) * 512], in_=pr[:],
                            func=AF.Relu, accum_out=rs[:, m, nq : nq + 1],
                        )

                # G factors: q = (S/100), prefix products P via DVE scan
                q = tiny.tile([128, TT], F32, tag="q")
                for m in range(TT):
                    rsum = tiny.tile([128, 1], F32, tag="rsum")
                    nc.vector.tensor_reduce(
                        out=rsum[:], in_=rs[:, m, :], axis=mybir.AxisListType.X, op=OP.add
                    )
                    nc.vector.tensor_scalar(
                        out=q[:, m : m + 1], in0=rsum[:], scalar1=svb[:, m : m + 1],
                        scalar2=0.01, op0=OP.add, op1=OP.mult,
                    )
                # q is on t-partitions; move to a row, scan for prefix
                # products P, and bring P and 1/q back to partitions with a
                # single full-width transpose (rows 0:2 = P, rows 2:4 = 1/q;
                # u = P_{s-1} = P_s * (1/q_s) applied as two scalar factors)
                pq = psS.tile([TT, 128], F32, tag="pss")
                nc.tensor.transpose(out=pq[:], in_=q[:], identity=identf[:])
                qrow = tiny.tile([TT, 128], F32, tag="qrow")
                nc.vector.tensor_copy(out=qrow[:], in_=pq[:])
                pad = scratch.tile([128, 128], F32, tag="pm1pad")
                nc.vector.tensor_tensor_scan(
                    out=pad[0:2, :], data0=qrow[:], data1=qrow[:], initial=1.0,
                    op0=OP.mult, op1=OP.bypass,
                )
                nc.vector.reciprocal(out=pad[32:34, :], in_=qrow[:])
                rP = tiny.tile([TT, 128], F32, tag="rP")
                nc.vector.reciprocal(out=rP[:], in_=pad[0:2, :])
                pb = psS.tile([128, 128], F32, tag="pss")
                nc.tensor.transpose(out=pb[:], in_=pad[:], identity=identf[:])
                Pq4 = tiny.tile([128, 4], F32, tag="Pq4")
                nc.vector.tensor_copy(out=Pq4[:, 0:2], in_=pb[:, 0:2])
                nc.vector.tensor_copy(out=Pq4[:, 2:4], in_=pb[:, 32:34])
                ptb = psS.tile([128, 1], F32, tag="pss")
                nc.tensor.matmul(ptb[:], esel[:, 0, :], rP[:, 127:128],
                                 start=True, stop=True)
                # ucross = P_{s-1}(tile0) / Ptot0 = ptb * P_s * (1/q_s)
                ucross = tiny.tile([128, 1], F32, tag="ucross")
                nc.vector.tensor_scalar(
                    out=ucross[:], in0=ptb[:], scalar1=Pq4[:, 0:1],
                    scalar2=Pq4[:, 2:3], op0=OP.mult, op1=OP.mult,
                )
                GT = mid.tile([128, TT, T], mm_dt, tag="GT")
                for tau in range(TT):
                    pw = psS.tile([128, 128], F32, tag="pss")
                    nc.tensor.matmul(pw[:], esel[:, tau, :], rP[:, :],
                                     start=True, stop=True)
                    for m in range(TT):
                        if tau == 0 and m == 1:
                            continue
                        tmp = scratch.tile([128, 128], F32, tag="gt_tmp")
                        if tau == 1 and m == 0:
                            nc.vector.tensor_scalar(
                                out=tmp[:], in0=gexp[:, m, 128:256],
                                scalar1=ucross[:], scalar2=None, op0=OP.mult,
                            )
                        else:
                            nc.vector.tensor_scalar(
                                out=tmp[:], in0=gexp[:, m, tau * 128 : (tau + 1) * 128],
                                scalar1=Pq4[:, m : m + 1],
                                scalar2=Pq4[:, 2 + m : 3 + m],
                                op0=OP.mult, op1=OP.mult,
                            )
                        nc.vector.tensor_tensor(
                            out=GT[:, m, tau * 128 : (tau + 1) * 128],
                            in0=tmp[:], in1=pw[:], op=OP.mult,
                        )
                tiles[b] = {"U": U, "R": R, "GT": GT}

            def stage_xt(b):
                """X^T = R^T @ G^T (2^8 scale); s-tile 1 only feeds t >= 128."""
                t = tiles[b]
                XT = big.tile([128, NT, T], mm_dt, tag="XT")
                t["XT"] = XT
                R, GT = t["R"], t["GT"]
                for ni in range(NT // 2):
                    px = psA.tile([128, 2, 256], F32, tag="psa")
                    for h in range(2):
                        nt = 2 * ni + h
                        nc.tensor.matmul(
                            px[:, h, :],
                            R[:, 0, nt * 128 : (nt + 1) * 128], GT[:, 0, :],
                            start=True, stop=False,
                        )
                        nc.tensor.matmul(
                            px[:, h, 128:256],
                            R[:, 1, nt * 128 : (nt + 1) * 128], GT[:, 1, 128:],
                            start=False, stop=True, skip_group_check=True,
                        )
                    nc.vector.tensor_copy(
                        out=XT[:, 2 * ni : 2 * ni + 2, :], in_=px[:]
                    )

            def stage_back(b):
                """Scores/AT, centered a*, y, v* = LN(y@E^T) -> out."""
                t = tiles.pop(b)
                XT, U = t["XT"], t["U"]

                AT = mid.tile([128, TT, T], mm_dt, tag="AT")
                for st in range(TT):
                    lo = st * 128
                    psc = psA.tile([128, T - lo], F32, tag="psa")
                    for k in range(NT):
                        nc.tensor.matmul(
                            psc[:], XT[:, k, lo : lo + 128], XT[:, k, lo:],
                            start=(k == 0), stop=(k == NT - 1),
                        )
                    nc.vector.tensor_tensor(
                        out=AT[:, st, lo:], in0=psc[:], in1=decayT[:, st, lo:], op=OP.mult
                    )

                ynorm = mid.tile([128, TT, D], F16, tag="ynorm")
                ynormT = mid.tile([128, DT, T], mm_dt, tag="ynormT")
                epsv = tiny.tile([128, TT], F32, tag="epsv")
                for tt in range(TT):
                    pa = psA.tile([128, D], F32, tag="psa")
                    for k in range(tt + 1):
                        nc.tensor.matmul(
                            pa[:], AT[:, k, tt * 128 : (tt + 1) * 128], U[:, k, :],
                            start=(k == 0), stop=(k == tt),
                        )
                    mv = _ln_stats(nc, tiny, pa[:])
                    sa = tiny.tile([128, 1], F32, tag="sa")
                    nc.scalar.activation(out=sa[:], in_=mv[:, 1:2], func=AF.Sqrt,
                                         scale=DDOF)
                    nc.vector.tensor_scalar(
                        out=epsv[:, tt : tt + 1], in0=sa[:], scalar1=EPSV_COEF,
                        scalar2=EPSV_CONST, op0=OP.mult, op1=OP.add,
                    )
                    nc.vector.tensor_scalar(
                        out=ynorm[:, tt, :], in0=pa[:], scalar1=mv[:, 0:1],
                        scalar2=None, op0=OP.subtract,
                    )
                    for kd in range(DT):
                        pt = psS.tile([128, 128], F16, tag="pss")
                        nc.tensor.transpose(
                            out=pt[:], in_=ynorm[:, tt, kd * 128 : (kd + 1) * 128],
                            identity=identh[:],
                        )
                        nc.vector.tensor_copy(
                            out=ynormT[:, kd, tt * 128 : (tt + 1) * 128], in_=pt[:]
                        )

                for ni in range(NT // 2):
                    py = psA.tile([128, 2, 256], F32, tag="psa")
                    for h in range(2):
                        nt = 2 * ni + h
                        for kd in range(DT):
                            nc.tensor.matmul(
                                py[:, h, :],
                                dyt[:, kd, nt * 128 : (nt + 1) * 128],
                                ynormT[:, kd, :],
                                start=(kd == 0), stop=(kd == DT - 1),
                            )
                    yr = scratch.tile([128, 2, 256], mm_dt, tag="yrelu")
                    nc.scalar.activation(out=yr[:], in_=py[:], func=AF.Relu)
                    xv = XT[:, 2 * ni : 2 * ni + 2, :]
                    nc.vector.tensor_tensor(out=xv, in0=yr[:], in1=xv, op=OP.mult)

                for tt in range(TT):
                    pv = psA.tile([128, D], F32, tag="psa")
                    for k in range(NT):
                        nc.tensor.matmul(
                            pv[:], XT[:, k, tt * 128 : (tt + 1) * 128], et[:, k, :],
                            start=(k == 0), stop=(k == NT - 1),
                        )
                    vstar = scratch.tile([128, D], F32, tag="vstar")
                    _ln_row(nc, tiny, pv[:], vstar[:], 0.0,
                            eps_ap=epsv[:, tt : tt + 1])
                    nc.sync.dma_start(
                        out=out_d[b, tt * 128 : (tt + 1) * 128, :], in_=vstar[:]
                    )

            # ---- software-pipelined emission ----
            stage_front(0)
            emit_late_consts()
            stage_front(1)
            for b in range(BL):
                stage_xt(b)
                stage_back(b)
                if b + 2 < BL:
                    stage_front(b + 2)

    nc.compile()
    return nc


_NC_CACHE = {}


def _get_nc(mm_dt=MM_DT):
    key = str(mm_dt)
    if key not in _NC_CACHE:
        _NC_CACHE[key] = build_nc(mm_dt)
    return _NC_CACHE[key]


def make_in_maps(idx, token_emb, E, Dx, Dy, mm_dt=MM_DT):
    wdt = mybir.dt.np(mm_dt)
    idx = np.ascontiguousarray(np.asarray(idx).astype(np.int32))
    temb = np.ascontiguousarray(np.asarray(token_emb, np.float32).astype(np.float16))
    dxt = np.ascontiguousarray(np.asarray(Dx, np.float32).T.astype(wdt))
    dyt = np.ascontiguousarray(np.asarray(Dy, np.float32).T.astype(wdt))
    et = np.ascontiguousarray(np.asarray(E, np.float32).T.astype(wdt))
    consts = _host_consts()
    identh = np.ascontiguousarray(np.eye(128, dtype=np.float16))
    identf = np.ascontiguousarray(np.eye(128, dtype=np.float32))
    esel = np.zeros((2, TT, 128), np.float32)
    for j in range(TT):
        esel[j, j, :] = 1.0
    shared = {"temb": temb, "dxt": dxt, "dyt": dyt, "et": et,
              "identh": identh, "identf": identf,
              "esel": np.ascontiguousarray(esel), **consts}
    in_maps = []
    for c in range(NCORES):
        m = dict(shared)
        m["idx"] = np.ascontiguousarray(idx[c * BL : (c + 1) * BL].reshape(-1))
        in_maps.append(m)
    return in_maps


def kernel(idx, token_emb, E, Dx, Dy):
    nc = _get_nc()
    in_maps = make_in_maps(idx, token_emb, E, Dx, Dy)
    res = run_bass_kernel_spmd(nc, in_maps, core_ids=list(range(NCORES)))
    out = np.concatenate([r["out"] for r in res.results], axis=0)
    return out
